# revision 2
# baseline (speedup 1.0000x reference)
"""Causal multi-head attention on 8 TRN2 NeuronCores.

Sharding: 8 cores = 4 batches x 2 head-groups (8 heads each).
Each core computes q/k/v projections for its head group, flash-style
causal attention in S^T layout ([k, q], softmax across partitions via a
ones-column in the PV matmul), and a partial output projection
(row-split Wo).  Host sums the two partial outputs per batch.

All matmuls run in bf16 with fp32 PSUM accumulation.  Activations are
fed to the device pre-transposed ([E, L]) and pre-tiled so every DMA is
contiguous.
"""

import sys

sys.path.insert(0, "/opt/trn_rl_repo")

from contextlib import ExitStack

import numpy as np
import ml_dtypes

import concourse.bass as bass
import concourse.mybir as mybir
import concourse.tile as tile
from concourse import bacc
from concourse.bass_utils import run_bass_kernel_spmd

BF16 = mybir.dt.bfloat16
F32 = mybir.dt.float32

B, L, E, H, D = 4, 2048, 1024, 16, 64
NCORES = 8
HPC = H // 2          # heads per core (8)
DH = HPC * D          # per-core projected dim (512)
LB = 512              # q-block width
NLB = L // LB         # 4
ET = E // 128         # 8 contraction tiles for projections
MT = DH // 128        # 4 dout tiles
KT = L // 128         # 16 key tiles
EXP_SCALE = 1.0 / np.sqrt(D)


def _build():
    nc = bacc.Bacc("TRN2", target_bir_lowering=False, debug=False,
                   num_devices=NCORES)

    qT_in = nc.dram_tensor("qT", [ET, NLB, 128, LB], BF16, kind="ExternalInput").ap()
    kT_in = nc.dram_tensor("kT", [ET, NLB, 128, LB], BF16, kind="ExternalInput").ap()
    vT_in = nc.dram_tensor("vT", [ET, NLB, 128, LB], BF16, kind="ExternalInput").ap()
    wq_in = nc.dram_tensor("wq", [ET, 128, DH], BF16, kind="ExternalInput").ap()
    wk_in = nc.dram_tensor("wk", [ET, 128, DH], BF16, kind="ExternalInput").ap()
    wv_in = nc.dram_tensor("wv", [ET, 128, DH], BF16, kind="ExternalInput").ap()
    wo_in = nc.dram_tensor("wo", [MT, 128, E], BF16, kind="ExternalInput").ap()
    bias_in = nc.dram_tensor("bias", [128, KT], F32, kind="ExternalInput").ap()
    out_ext = nc.dram_tensor("out", [KT, 2, 128, LB], BF16, kind="ExternalOutput").ap()

    with tile.TileContext(nc) as tc, ExitStack() as ctx:
        wpool = ctx.enter_context(tc.tile_pool(name="weights", bufs=1))
        ppool = ctx.enter_context(tc.tile_pool(name="persist", bufs=1))
        xpool = ctx.enter_context(tc.tile_pool(name="xT", bufs=10))
        pTpool = ctx.enter_context(tc.tile_pool(name="pT", bufs=3))
        opool = ctx.enter_context(tc.tile_pool(name="outsb", bufs=3))
        rpool = ctx.enter_context(tc.tile_pool(name="rnorm", bufs=4))
        ps_proj = ctx.enter_context(tc.tile_pool(name="ps_proj", bufs=2, space="PSUM"))
        ps_sp = ctx.enter_context(tc.tile_pool(name="ps_sp", bufs=2, space="PSUM"))
        ps_oacc = ctx.enter_context(tc.tile_pool(name="ps_oacc", bufs=2, space="PSUM"))

        # ---- resident weights -------------------------------------------
        wq_sb = wpool.tile([128, ET, DH], BF16, tag="wq")
        wk_sb = wpool.tile([128, ET, DH], BF16, tag="wk")
        wv_sb = wpool.tile([128, ET, DH], BF16, tag="wv")
        wo_sb = wpool.tile([128, MT, E], BF16, tag="wo")
        for t in range(ET):
            nc.sync.dma_start(wq_sb[:, t, :], wq_in[t])
            nc.sync.dma_start(wk_sb[:, t, :], wk_in[t])
            nc.sync.dma_start(wv_sb[:, t, :], wv_in[t])
        for r in range(MT):
            nc.sync.dma_start(wo_sb[:, r, :], wo_in[r])
        bias_sb = wpool.tile([128, KT], F32, tag="bias")
        nc.sync.dma_start(bias_sb[:], bias_in[:])

        # ---- resident activations ---------------------------------------
        qT_sb = ppool.tile([128, MT, L], BF16, tag="qT")
        kT_sb = ppool.tile([128, MT, L], BF16, tag="kT")
        v_sb = ppool.tile([128, KT, HPC, D + 1], BF16, tag="v")
        oT_sb = ppool.tile([128, MT, L], BF16, tag="oT")
        nc.gpsimd.memset(v_sb[:, :, :, D:D + 1], 1.0)

        # ---- pipeline units ---------------------------------------------
        def proj_block(lb):
            xq, xk, xv = [], [], []
            for t in range(ET):
                tq = xpool.tile([128, LB], BF16, tag="xq")
                nc.sync.dma_start(tq[:], qT_in[t, lb])
                xq.append(tq)
                tk = xpool.tile([128, LB], BF16, tag="xk")
                nc.sync.dma_start(tk[:], kT_in[t, lb])
                xk.append(tk)
                tv = xpool.tile([128, LB], BF16, tag="xv")
                nc.sync.dma_start(tv[:], vT_in[t, lb])
                xv.append(tv)
            for dst, xs, w_sb in ((qT_sb, xq, wq_sb), (kT_sb, xk, wk_sb)):
                for m in range(MT):
                    ps = ps_proj.tile([128, LB], F32, tag="ps_proj")
                    for t in range(ET):
                        nc.tensor.matmul(
                            ps[:],
                            lhsT=w_sb[:, t, m * 128:(m + 1) * 128],
                            rhs=xs[t][:],
                            start=(t == 0), stop=(t == ET - 1))
                    nc.vector.tensor_copy(dst[:, m, lb * LB:(lb + 1) * LB], ps[:])
            for lt in range(LB // 128):
                ps = ps_proj.tile([128, HPC, D], F32, tag="ps_proj")
                for t in range(ET):
                    nc.tensor.matmul(
                        ps[:],
                        lhsT=xv[t][:, lt * 128:(lt + 1) * 128],
                        rhs=wv_sb[:, t, :],
                        start=(t == 0), stop=(t == ET - 1))
                nc.vector.tensor_copy(v_sb[:, lb * 4 + lt, :, 0:D], ps[:])

        def attn_pair(hp, j):
            nki = 4 * j + 4
            oaccs = [ps_oacc.tile([D + 1, LB], F32, tag="oacc", name="oacc") for _ in range(2)]
            for ki in range(nki):
                sp = ps_sp.tile([128, 2, LB], F32, tag="sp")
                for hi in range(2):
                    p0 = hi * 64
                    nc.tensor.matmul(
                        sp[:, hi, :],
                        lhsT=kT_sb[p0:p0 + 64, hp, ki * 128:(ki + 1) * 128],
                        rhs=qT_sb[p0:p0 + 64, hp, j * LB:(j + 1) * LB],
                        start=True, stop=True, tile_position=(p0, 0))
                pT = pTpool.tile([128, 2, LB], BF16, tag="pT")
                nc.scalar.activation(pT[:], sp[:],
                                     mybir.ActivationFunctionType.Exp,
                                     bias=bias_sb[:, ki:ki + 1],
                                     scale=float(EXP_SCALE))
                if ki >= 4 * j:
                    # diagonal tile: zero entries with q < k after the exp
                    nc.gpsimd.affine_select(
                        out=pT[:], in_=pT[:],
                        compare_op=mybir.AluOpType.is_ge,
                        fill=0.0,
                        base=(j * LB - ki * 128),
                        pattern=[[0, 2], [1, LB]],
                        channel_multiplier=-1)
                for hi in range(2):
                    nc.tensor.matmul(
                        oaccs[hi][:],
                        lhsT=v_sb[:, ki, 2 * hp + hi, :],
                        rhs=pT[:, hi, :],
                        start=(ki == 0), stop=(ki == nki - 1))
            for hi in range(2):
                r_inv = rpool.tile([1, LB], F32, tag="rinv")
                nc.vector.reciprocal(r_inv[:], oaccs[hi][D:D + 1, :])
                rb = rpool.tile([64, LB], F32, tag="rb")
                nc.gpsimd.partition_broadcast(rb[:], r_inv[:])
                p0 = hi * 64
                nc.vector.tensor_mul(
                    oT_sb[p0:p0 + 64, hp, j * LB:(j + 1) * LB],
                    oaccs[hi][0:D, :], rb[:])

        def oproj_block(lb):
            for lt in range(LB // 128):
                l_tile = lb * 4 + lt
                for e in range(2):
                    ps = ps_proj.tile([128, LB], F32, tag="ps_proj")
                    for r in range(MT):
                        nc.tensor.matmul(
                            ps[:],
                            lhsT=oT_sb[:, r, l_tile * 128:(l_tile + 1) * 128],
                            rhs=wo_sb[:, r, e * LB:(e + 1) * LB],
                            start=(r == 0), stop=(r == MT - 1))
                    ob = opool.tile([128, LB], BF16, tag="ob")
                    nc.vector.tensor_copy(ob[:], ps[:])
                    nc.sync.dma_start(out_ext[l_tile, e], ob[:])

        proj_block(0)
        for j in range(NLB):
            for hp in range(HPC // 2):
                attn_pair(hp, j)
            if j + 1 < NLB:
                proj_block(j + 1)
            oproj_block(j)

    nc.compile()
    return nc


_CACHE = {}


def _get_nc():
    if "nc" not in _CACHE:
        _CACHE["nc"] = _build()
    return _CACHE["nc"]


def _prepare_in_maps(query, key, value, pad_mask, Wq, Wk, Wv, Wo):
    bf = ml_dtypes.bfloat16
    query = np.asarray(query, np.float32)
    key = np.asarray(key, np.float32)
    value = np.asarray(value, np.float32)
    pad_mask = np.asarray(pad_mask)
    Wq = np.asarray(Wq, np.float32)
    Wk = np.asarray(Wk, np.float32)
    Wv = np.asarray(Wv, np.float32)
    Wo = np.asarray(Wo, np.float32)

    def tile_act(x):
        # [L, E] -> transposed + tiled [ET, NLB, 128, LB]
        xt = x.T.reshape(ET, 128, NLB, LB).transpose(0, 2, 1, 3)
        return np.ascontiguousarray(xt.astype(bf))

    per_batch = []
    for b in range(B):
        bias = np.where(pad_mask[b] != 0, 0.0, -30000.0).astype(np.float32)
        bias = np.ascontiguousarray(bias.reshape(KT, 128).T)
        per_batch.append({
            "qT": tile_act(query[b]),
            "kT": tile_act(key[b]),
            "vT": tile_act(value[b]),
            "bias": bias,
        })

    per_group = []
    for g in range(2):
        sl = slice(g * DH, (g + 1) * DH)
        per_group.append({
            "wq": np.ascontiguousarray(Wq[:, sl].astype(bf).reshape(ET, 128, DH)),
            "wk": np.ascontiguousarray(Wk[:, sl].astype(bf).reshape(ET, 128, DH)),
            "wv": np.ascontiguousarray(Wv[:, sl].astype(bf).reshape(ET, 128, DH)),
            "wo": np.ascontiguousarray(Wo[sl, :].astype(bf).reshape(MT, 128, E)),
        })

    in_maps = []
    for b in range(B):
        for g in range(2):
            m = dict(per_batch[b])
            m.update(per_group[g])
            in_maps.append(m)
    return in_maps


def _combine(results):
    out = np.empty((B, L, E), np.float32)
    for b in range(B):
        acc = (results[2 * b]["out"].astype(np.float32)
               + results[2 * b + 1]["out"].astype(np.float32))
        out[b] = acc.transpose(0, 2, 1, 3).reshape(L, E)
    return out


def kernel(query, key, value, pad_mask, Wq, Wk, Wv, Wo):
    nc = _get_nc()
    in_maps = _prepare_in_maps(query, key, value, pad_mask, Wq, Wk, Wv, Wo)
    res = run_bass_kernel_spmd(nc, in_maps, core_ids=list(range(NCORES)))
    return _combine(res.results)


# revision 6
# speedup vs baseline: 1.1044x; 1.1044x over previous
"""Causal multi-head attention on 8 TRN2 NeuronCores.

Sharding: 8 cores = 4 batches x 2 head-groups (8 heads each).
Each core computes q/k/v projections for its head group, flash-style
causal attention in S^T layout ([k, q], softmax across partitions via a
ones-column in the PV matmul), and a partial output projection
(row-split Wo).  Host sums the two partial outputs per batch.

All matmuls run in bf16 with fp32 PSUM accumulation.  Activations are
fed to the device pre-transposed ([E, L]) and pre-tiled so every DMA
moves >=4KB contiguous per partition.
"""

import sys

sys.path.insert(0, "/opt/trn_rl_repo")

from contextlib import ExitStack

import numpy as np
import ml_dtypes

import concourse.bass as bass
import concourse.mybir as mybir
import concourse.tile as tile
from concourse import bacc
from concourse.bass_utils import run_bass_kernel_spmd

BF16 = mybir.dt.bfloat16
F32 = mybir.dt.float32

B, L, E, H, D = 4, 2048, 1024, 16, 64
NCORES = 8
HPC = H // 2          # heads per core (8)
DH = HPC * D          # per-core projected dim (512)
LB = 512              # q-block width
NLB = L // LB         # 4
ET = E // 128         # 8 contraction tiles for projections
EG = 2                # e-tile groups per DMA (ET // 4)
MT = DH // 128        # 4 dout tiles
KT = L // 128         # 16 key tiles
EXP_SCALE = 1.0 / np.sqrt(D)


def _build():
    nc = bacc.Bacc("TRN2", target_bir_lowering=False, debug=False,
                   num_devices=NCORES)

    qT_in = nc.dram_tensor("qT", [EG, NLB, 128, ET // EG, LB], BF16, kind="ExternalInput").ap()
    kT_in = nc.dram_tensor("kT", [EG, NLB, 128, ET // EG, LB], BF16, kind="ExternalInput").ap()
    vT_in = nc.dram_tensor("vT", [EG, NLB, 128, ET // EG, LB], BF16, kind="ExternalInput").ap()
    wq_in = nc.dram_tensor("wq", [128, ET, DH], BF16, kind="ExternalInput").ap()
    wk_in = nc.dram_tensor("wk", [128, ET, DH], BF16, kind="ExternalInput").ap()
    wv_in = nc.dram_tensor("wv", [128, ET, DH], BF16, kind="ExternalInput").ap()
    wo_in = nc.dram_tensor("wo", [128, MT, E], BF16, kind="ExternalInput").ap()
    bias_in = nc.dram_tensor("bias", [128, KT], F32, kind="ExternalInput").ap()
    out_ext = nc.dram_tensor("out", [KT, 128, 2, LB], BF16, kind="ExternalOutput").ap()

    with tile.TileContext(nc) as tc, ExitStack() as ctx:
        wpool = ctx.enter_context(tc.tile_pool(name="weights", bufs=1))
        ppool = ctx.enter_context(tc.tile_pool(name="persist", bufs=1))
        xpool = ctx.enter_context(tc.tile_pool(name="xT", bufs=3))
        pTpool = ctx.enter_context(tc.tile_pool(name="pT", bufs=3))
        opool = ctx.enter_context(tc.tile_pool(name="outsb", bufs=3))
        rpool = ctx.enter_context(tc.tile_pool(name="rnorm", bufs=3))
        ps_proj = ctx.enter_context(tc.tile_pool(name="ps_proj", bufs=2, space="PSUM"))
        ps_sp = ctx.enter_context(tc.tile_pool(name="ps_sp", bufs=2, space="PSUM"))
        ps_oacc = ctx.enter_context(tc.tile_pool(name="ps_oacc", bufs=2, space="PSUM"))

        # ---- resident weights (one DMA each, >=4KB/partition) ------------
        wq_sb = wpool.tile([128, ET, DH], BF16, tag="wq")
        wk_sb = wpool.tile([128, ET, DH], BF16, tag="wk")
        wv_sb = wpool.tile([128, ET, DH], BF16, tag="wv")
        wo_sb = wpool.tile([128, MT, E], BF16, tag="wo")
        nc.sync.dma_start(wq_sb[:], wq_in[:])
        nc.sync.dma_start(wk_sb[:], wk_in[:])
        nc.sync.dma_start(wv_sb[:], wv_in[:])
        nc.sync.dma_start(wo_sb[:], wo_in[:])
        bias_sb = wpool.tile([128, KT], F32, tag="bias")
        nc.sync.dma_start(bias_sb[:], bias_in[:])

        # ---- resident activations ---------------------------------------
        qT_sb = ppool.tile([128, MT, L], BF16, tag="qT")
        kT_sb = ppool.tile([128, MT, L], BF16, tag="kT")
        v_sb = ppool.tile([128, KT, HPC, D + 1], BF16, tag="v")
        oT_sb = ppool.tile([128, MT, L], BF16, tag="oT")
        nc.gpsimd.memset(v_sb[:, :, :, D:D + 1], 1.0)

        # ---- pipeline units ---------------------------------------------
        def proj_block(lb):
            xq, xk, xv = [], [], []
            for eg in range(EG):
                tq = xpool.tile([128, ET // EG, LB], BF16, tag="xq")
                nc.sync.dma_start(tq[:], qT_in[eg, lb])
                xq.append(tq)
                tk = xpool.tile([128, ET // EG, LB], BF16, tag="xk")
                nc.sync.dma_start(tk[:], kT_in[eg, lb])
                xk.append(tk)
                tv = xpool.tile([128, ET // EG, LB], BF16, tag="xv")
                nc.sync.dma_start(tv[:], vT_in[eg, lb])
                xv.append(tv)
            for dst, xs, w_sb in ((qT_sb, xq, wq_sb), (kT_sb, xk, wk_sb)):
                for m in range(MT):
                    ps = ps_proj.tile([128, LB], F32, tag="ps_proj")
                    for t in range(ET):
                        nc.tensor.matmul(
                            ps[:],
                            lhsT=w_sb[:, t, m * 128:(m + 1) * 128],
                            rhs=xs[t // 4][:, t % 4, :],
                            start=(t == 0), stop=(t == ET - 1))
                    nc.vector.tensor_copy(dst[:, m, lb * LB:(lb + 1) * LB], ps[:])
            for lt in range(LB // 128):
                ps = ps_proj.tile([128, HPC, D], F32, tag="ps_proj")
                for t in range(ET):
                    nc.tensor.matmul(
                        ps[:],
                        lhsT=xv[t // 4][:, t % 4, lt * 128:(lt + 1) * 128],
                        rhs=wv_sb[:, t, :],
                        start=(t == 0), stop=(t == ET - 1))
                nc.vector.tensor_copy(v_sb[:, lb * 4 + lt, :, 0:D], ps[:])

        def attn_pair(hp, j):
            nki = 4 * j + 4
            oaccs = [ps_oacc.tile([D + 1, LB], F32, tag="oacc", name="oacc")
                     for _ in range(2)]
            for ki in range(nki):
                s = ki - 4 * j
                sp = ps_sp.tile([128, 2, LB], F32, tag="sp")
                for hi in range(2):
                    p0 = hi * 64
                    nc.tensor.matmul(
                        sp[:, hi, :],
                        lhsT=kT_sb[p0:p0 + 64, hp, ki * 128:(ki + 1) * 128],
                        rhs=qT_sb[p0:p0 + 64, hp, j * LB:(j + 1) * LB],
                        start=True, stop=True, tile_position=(p0, 0))
                pT = pTpool.tile([128, 2, LB], BF16, tag="pT")
                nc.scalar.activation(pT[:], sp[:],
                                     mybir.ActivationFunctionType.Exp,
                                     bias=bias_sb[:, ki:ki + 1],
                                     scale=float(EXP_SCALE))
                if s >= 0:
                    nc.gpsimd.affine_select(
                        out=pT[:], in_=pT[:],
                        compare_op=mybir.AluOpType.is_ge,
                        fill=0.0,
                        base=(j * LB - ki * 128),
                        pattern=[[0, 2], [1, LB]],
                        channel_multiplier=-1)
                for hi in range(2):
                    nc.tensor.matmul(
                        oaccs[hi][:],
                        lhsT=v_sb[:, ki, 2 * hp + hi, :],
                        rhs=pT[:, hi, :],
                        start=(ki == 0), stop=(ki == nki - 1))
            for hi in range(2):
                osb = rpool.tile([D + 1, LB], F32, tag="osb")
                nc.vector.tensor_copy(osb[:], oaccs[hi][:])
                rinv1 = rpool.tile([1, LB], F32, tag="rinv1")
                nc.vector.reciprocal(rinv1[:], osb[D:D + 1, :])
                rinv = rpool.tile([64, LB], F32, tag="rinv")
                nc.gpsimd.partition_broadcast(rinv[:], rinv1[:])
                p0 = hi * 64
                nc.vector.tensor_mul(
                    oT_sb[p0:p0 + 64, hp, j * LB:(j + 1) * LB],
                    osb[0:D, :], rinv[:])

        def oproj_block(lb):
            for lt in range(LB // 128):
                l_tile = lb * 4 + lt
                ob = opool.tile([128, 2, LB], BF16, tag="ob")
                for e in range(2):
                    ps = ps_proj.tile([128, LB], F32, tag="ps_proj")
                    for r in range(MT):
                        nc.tensor.matmul(
                            ps[:],
                            lhsT=oT_sb[:, r, l_tile * 128:(l_tile + 1) * 128],
                            rhs=wo_sb[:, r, e * LB:(e + 1) * LB],
                            start=(r == 0), stop=(r == MT - 1))
                    nc.vector.tensor_copy(ob[:, e, :], ps[:])
                nc.sync.dma_start(out_ext[l_tile], ob[:])

        proj_block(0)
        for j in range(NLB):
            for hp in range(HPC // 2):
                attn_pair(hp, j)
            if j + 1 < NLB:
                proj_block(j + 1)
            oproj_block(j)

    nc.compile()
    return nc


_CACHE = {}


def _get_nc():
    if "nc" not in _CACHE:
        _CACHE["nc"] = _build()
    return _CACHE["nc"]


def _prepare_in_maps(query, key, value, pad_mask, Wq, Wk, Wv, Wo):
    bf = ml_dtypes.bfloat16
    query = np.asarray(query, np.float32)
    key = np.asarray(key, np.float32)
    value = np.asarray(value, np.float32)
    pad_mask = np.asarray(pad_mask)
    Wq = np.asarray(Wq, np.float32)
    Wk = np.asarray(Wk, np.float32)
    Wv = np.asarray(Wv, np.float32)
    Wo = np.asarray(Wo, np.float32)

    def tile_act(x):
        # [L, E] -> [E, L] -> [EG, NLB, 128, ET//EG, LB], 4KB/partition chunks
        xt = x.T.reshape(EG, ET // EG, 128, NLB, LB).transpose(0, 3, 2, 1, 4)
        return np.ascontiguousarray(xt.astype(bf))

    per_batch = []
    for b in range(B):
        bias = np.where(pad_mask[b] != 0, 0.0, -30000.0).astype(np.float32)
        bias = np.ascontiguousarray(bias.reshape(KT, 128).T)
        per_batch.append({
            "qT": tile_act(query[b]),
            "kT": tile_act(key[b]),
            "vT": tile_act(value[b]),
            "bias": bias,
        })

    per_group = []
    for g in range(2):
        sl = slice(g * DH, (g + 1) * DH)
        per_group.append({
            "wq": np.ascontiguousarray(
                Wq[:, sl].astype(bf).reshape(ET, 128, DH).transpose(1, 0, 2)),
            "wk": np.ascontiguousarray(
                Wk[:, sl].astype(bf).reshape(ET, 128, DH).transpose(1, 0, 2)),
            "wv": np.ascontiguousarray(
                Wv[:, sl].astype(bf).reshape(ET, 128, DH).transpose(1, 0, 2)),
            "wo": np.ascontiguousarray(
                Wo[sl, :].astype(bf).reshape(MT, 128, E).transpose(1, 0, 2)),
        })

    in_maps = []
    for b in range(B):
        for g in range(2):
            m = dict(per_batch[b])
            m.update(per_group[g])
            in_maps.append(m)
    return in_maps


def _combine(results):
    out = np.empty((B, L, E), np.float32)
    for b in range(B):
        acc = (results[2 * b]["out"].astype(np.float32)
               + results[2 * b + 1]["out"].astype(np.float32))
        out[b] = acc.reshape(L, E)
    return out


def kernel(query, key, value, pad_mask, Wq, Wk, Wv, Wo):
    nc = _get_nc()
    in_maps = _prepare_in_maps(query, key, value, pad_mask, Wq, Wk, Wv, Wo)
    res = run_bass_kernel_spmd(nc, in_maps, core_ids=list(range(NCORES)))
    return _combine(res.results)


# revision 8
# speedup vs baseline: 1.1889x; 1.0765x over previous
"""Causal multi-head attention on 8 TRN2 NeuronCores.

Sharding: 8 cores = 4 batches x 2 head-groups (8 heads each).
Each core computes q/k/v projections for its head group, flash-style
causal attention in S^T layout ([k, q], softmax across partitions via a
ones-column in the PV matmul), and a partial output projection
(row-split Wo).  Host sums the two partial outputs per batch.

All matmuls run in bf16 with fp32 PSUM accumulation.  Activations are
fed to the device pre-transposed ([E, L]) and pre-tiled so every DMA
moves >=4KB contiguous per partition.
"""

import sys

sys.path.insert(0, "/opt/trn_rl_repo")

from contextlib import ExitStack

import numpy as np
import ml_dtypes

import concourse.bass as bass
import concourse.mybir as mybir
import concourse.tile as tile
from concourse import bacc
from concourse.bass_utils import run_bass_kernel_spmd

BF16 = mybir.dt.bfloat16
F32 = mybir.dt.float32

B, L, E, H, D = 4, 2048, 1024, 16, 64
NCORES = 8
HPC = H // 2          # heads per core (8)
DH = HPC * D          # per-core projected dim (512)
LB = 512              # q-block width
NLB = L // LB         # 4
ET = E // 128         # 8 contraction tiles for projections
EG = 2                # e-tile groups per DMA (ET // 4)
MT = DH // 128        # 4 dout tiles
KT = L // 128         # 16 key tiles
EXP_SCALE = 1.0 / np.sqrt(D)


def _build():
    nc = bacc.Bacc("TRN2", target_bir_lowering=False, debug=False,
                   num_devices=NCORES)

    qT_in = nc.dram_tensor("qT", [EG, NLB, 128, ET // EG, LB], BF16, kind="ExternalInput").ap()
    kT_in = nc.dram_tensor("kT", [EG, NLB, 128, ET // EG, LB], BF16, kind="ExternalInput").ap()
    vT_in = nc.dram_tensor("vT", [EG, NLB, 128, ET // EG, LB], BF16, kind="ExternalInput").ap()
    wq_in = nc.dram_tensor("wq", [128, ET, DH], BF16, kind="ExternalInput").ap()
    wk_in = nc.dram_tensor("wk", [128, ET, DH], BF16, kind="ExternalInput").ap()
    wv_in = nc.dram_tensor("wv", [128, ET, DH], BF16, kind="ExternalInput").ap()
    wo_in = nc.dram_tensor("wo", [128, MT, E], BF16, kind="ExternalInput").ap()
    bias_in = nc.dram_tensor("bias", [128, KT], F32, kind="ExternalInput").ap()
    out_ext = nc.dram_tensor("out", [KT, 128, 2, LB], BF16, kind="ExternalOutput").ap()

    with tile.TileContext(nc) as tc, ExitStack() as ctx:
        wpool = ctx.enter_context(tc.tile_pool(name="weights", bufs=1))
        ppool = ctx.enter_context(tc.tile_pool(name="persist", bufs=1))
        xpool = ctx.enter_context(tc.tile_pool(name="xT", bufs=3))
        pTpool = ctx.enter_context(tc.tile_pool(name="pT", bufs=3))
        opool = ctx.enter_context(tc.tile_pool(name="outsb", bufs=3))
        rpool = ctx.enter_context(tc.tile_pool(name="rnorm", bufs=4))
        ps_proj = ctx.enter_context(tc.tile_pool(name="ps_proj", bufs=2, space="PSUM"))
        ps_sp = ctx.enter_context(tc.tile_pool(name="ps_sp", bufs=2, space="PSUM"))
        ps_oacc = ctx.enter_context(tc.tile_pool(name="ps_oacc", bufs=2, space="PSUM"))

        # ---- resident weights (one DMA each, >=4KB/partition) ------------
        wq_sb = wpool.tile([128, ET, DH], BF16, tag="wq")
        wk_sb = wpool.tile([128, ET, DH], BF16, tag="wk")
        wv_sb = wpool.tile([128, ET, DH], BF16, tag="wv")
        wo_sb = wpool.tile([128, MT, E], BF16, tag="wo")
        nc.sync.dma_start(wq_sb[:], wq_in[:])
        nc.sync.dma_start(wk_sb[:], wk_in[:])
        nc.sync.dma_start(wv_sb[:], wv_in[:])
        nc.sync.dma_start(wo_sb[:], wo_in[:])
        bias_sb = wpool.tile([128, KT], F32, tag="bias")
        nc.sync.dma_start(bias_sb[:], bias_in[:])

        # ---- resident activations ---------------------------------------
        qT_sb = ppool.tile([128, MT, L], BF16, tag="qT")
        kT_sb = ppool.tile([128, MT, L], BF16, tag="kT")
        v_sb = ppool.tile([128, KT, HPC, D + 1], BF16, tag="v")
        oT_sb = ppool.tile([128, MT, L], BF16, tag="oT")
        nc.gpsimd.memset(v_sb[:, :, :, D:D + 1], 1.0)

        # ---- pipeline units ---------------------------------------------
        def proj_block(lb):
            xq, xk, xv = [], [], []
            for eg in range(EG):
                tq = xpool.tile([128, ET // EG, LB], BF16, tag="xq")
                nc.sync.dma_start(tq[:], qT_in[eg, lb])
                xq.append(tq)
                tk = xpool.tile([128, ET // EG, LB], BF16, tag="xk")
                nc.sync.dma_start(tk[:], kT_in[eg, lb])
                xk.append(tk)
                tv = xpool.tile([128, ET // EG, LB], BF16, tag="xv")
                nc.sync.dma_start(tv[:], vT_in[eg, lb])
                xv.append(tv)
            for dst, xs, w_sb in ((qT_sb, xq, wq_sb), (kT_sb, xk, wk_sb)):
                for m in range(MT):
                    ps = ps_proj.tile([128, LB], F32, tag="ps_proj")
                    for t in range(ET):
                        nc.tensor.matmul(
                            ps[:],
                            lhsT=w_sb[:, t, m * 128:(m + 1) * 128],
                            rhs=xs[t // 4][:, t % 4, :],
                            start=(t == 0), stop=(t == ET - 1))
                    nc.vector.tensor_copy(dst[:, m, lb * LB:(lb + 1) * LB], ps[:])
            for lt in range(LB // 128):
                ps = ps_proj.tile([128, HPC, D], F32, tag="ps_proj")
                for t in range(ET):
                    nc.tensor.matmul(
                        ps[:],
                        lhsT=xv[t // 4][:, t % 4, lt * 128:(lt + 1) * 128],
                        rhs=wv_sb[:, t, :],
                        start=(t == 0), stop=(t == ET - 1))
                nc.vector.tensor_copy(v_sb[:, lb * 4 + lt, :, 0:D], ps[:])

        def attn_pair(hp, j):
            nki = 4 * j + 4
            oaccs = [ps_oacc.tile([D + 1, LB], F32, tag="oacc", name="oacc")
                     for _ in range(2)]
            for ki in range(nki):
                s = ki - 4 * j
                x0 = 128 * s if s >= 0 else 0   # first causal-valid q column
                sp = ps_sp.tile([128, 2, LB], F32, tag="sp")
                for hi in range(2):
                    p0 = hi * 64
                    nc.tensor.matmul(
                        sp[:, hi, x0:LB],
                        lhsT=kT_sb[p0:p0 + 64, hp, ki * 128:(ki + 1) * 128],
                        rhs=qT_sb[p0:p0 + 64, hp, j * LB + x0:(j + 1) * LB],
                        start=True, stop=True, tile_position=(p0, 0))
                pT = pTpool.tile([128, 2, LB], BF16, tag="pT")
                nc.scalar.activation(pT[:, :, x0:LB], sp[:, :, x0:LB],
                                     mybir.ActivationFunctionType.Exp,
                                     bias=bias_sb[:, ki:ki + 1],
                                     scale=float(EXP_SCALE))
                if s >= 0:
                    # zero q < k inside the first 128 valid columns
                    nc.gpsimd.affine_select(
                        out=pT[:, :, x0:x0 + 128], in_=pT[:, :, x0:x0 + 128],
                        compare_op=mybir.AluOpType.is_ge,
                        fill=0.0,
                        base=0,
                        pattern=[[0, 2], [1, 128]],
                        channel_multiplier=-1)
                for hi in range(2):
                    nc.tensor.matmul(
                        oaccs[hi][:, x0:LB],
                        lhsT=v_sb[:, ki, 2 * hp + hi, :],
                        rhs=pT[:, hi, x0:LB],
                        start=(ki == 0), stop=(ki == nki - 1))
            osbs = []
            for hi in range(2):
                osb = rpool.tile([D + 1, LB], F32, tag="osb", name="osb")
                nc.vector.tensor_copy(osb[:], oaccs[hi][:])
                osbs.append(osb)

            def norm_fn():
                for hi in range(2):
                    osb = osbs[hi]
                    rinv1 = rpool.tile([1, LB], F32, tag="rinv1")
                    nc.vector.reciprocal(rinv1[:], osb[D:D + 1, :])
                    rinv = rpool.tile([64, LB], F32, tag="rinv")
                    nc.gpsimd.partition_broadcast(rinv[:], rinv1[:])
                    p0 = hi * 64
                    nc.vector.tensor_mul(
                        oT_sb[p0:p0 + 64, hp, j * LB:(j + 1) * LB],
                        osb[0:D, :], rinv[:])
            return norm_fn

        def oproj_block(lb):
            for lt in range(LB // 128):
                l_tile = lb * 4 + lt
                ob = opool.tile([128, 2, LB], BF16, tag="ob")
                for e in range(2):
                    ps = ps_proj.tile([128, LB], F32, tag="ps_proj")
                    for r in range(MT):
                        nc.tensor.matmul(
                            ps[:],
                            lhsT=oT_sb[:, r, l_tile * 128:(l_tile + 1) * 128],
                            rhs=wo_sb[:, r, e * LB:(e + 1) * LB],
                            start=(r == 0), stop=(r == MT - 1))
                    nc.vector.tensor_copy(ob[:, e, :], ps[:])
                nc.sync.dma_start(out_ext[l_tile], ob[:])

        proj_block(0)
        pending_norm = None
        for j in range(NLB):
            for hp in range(HPC // 2):
                nf = attn_pair(hp, j)
                if pending_norm is not None:
                    pending_norm()
                pending_norm = nf
            # flush before oproj needs oT of this j-block
            pending_norm()
            pending_norm = None
            if j + 1 < NLB:
                proj_block(j + 1)
            oproj_block(j)

    nc.compile()
    return nc


_CACHE = {}


def _get_nc():
    if "nc" not in _CACHE:
        _CACHE["nc"] = _build()
    return _CACHE["nc"]


def _prepare_in_maps(query, key, value, pad_mask, Wq, Wk, Wv, Wo):
    bf = ml_dtypes.bfloat16
    query = np.asarray(query, np.float32)
    key = np.asarray(key, np.float32)
    value = np.asarray(value, np.float32)
    pad_mask = np.asarray(pad_mask)
    Wq = np.asarray(Wq, np.float32)
    Wk = np.asarray(Wk, np.float32)
    Wv = np.asarray(Wv, np.float32)
    Wo = np.asarray(Wo, np.float32)

    def tile_act(x):
        # [L, E] -> [E, L] -> [EG, NLB, 128, ET//EG, LB], 4KB/partition chunks
        xt = x.T.reshape(EG, ET // EG, 128, NLB, LB).transpose(0, 3, 2, 1, 4)
        return np.ascontiguousarray(xt.astype(bf))

    per_batch = []
    for b in range(B):
        bias = np.where(pad_mask[b] != 0, 0.0, -30000.0).astype(np.float32)
        bias = np.ascontiguousarray(bias.reshape(KT, 128).T)
        per_batch.append({
            "qT": tile_act(query[b]),
            "kT": tile_act(key[b]),
            "vT": tile_act(value[b]),
            "bias": bias,
        })

    per_group = []
    for g in range(2):
        sl = slice(g * DH, (g + 1) * DH)
        per_group.append({
            "wq": np.ascontiguousarray(
                Wq[:, sl].astype(bf).reshape(ET, 128, DH).transpose(1, 0, 2)),
            "wk": np.ascontiguousarray(
                Wk[:, sl].astype(bf).reshape(ET, 128, DH).transpose(1, 0, 2)),
            "wv": np.ascontiguousarray(
                Wv[:, sl].astype(bf).reshape(ET, 128, DH).transpose(1, 0, 2)),
            "wo": np.ascontiguousarray(
                Wo[sl, :].astype(bf).reshape(MT, 128, E).transpose(1, 0, 2)),
        })

    in_maps = []
    for b in range(B):
        for g in range(2):
            m = dict(per_batch[b])
            m.update(per_group[g])
            in_maps.append(m)
    return in_maps


def _combine(results):
    out = np.empty((B, L, E), np.float32)
    for b in range(B):
        acc = (results[2 * b]["out"].astype(np.float32)
               + results[2 * b + 1]["out"].astype(np.float32))
        out[b] = acc.reshape(L, E)
    return out


def kernel(query, key, value, pad_mask, Wq, Wk, Wv, Wo):
    nc = _get_nc()
    in_maps = _prepare_in_maps(query, key, value, pad_mask, Wq, Wk, Wv, Wo)
    res = run_bass_kernel_spmd(nc, in_maps, core_ids=list(range(NCORES)))
    return _combine(res.results)


# revision 12
# speedup vs baseline: 1.4093x; 1.1854x over previous
"""Causal multi-head attention on 8 TRN2 NeuronCores.

Sharding: 8 cores = 4 batches x 2 head-groups (8 heads each).
Each core computes q/k/v projections for its head group, flash-style
causal attention in S^T layout ([k, q], softmax across partitions via a
ones-column in the PV matmul), and a partial output projection
(row-split Wo).  Host sums the two partial outputs per batch.

All matmuls run in bf16 with fp32 PSUM accumulation.  Activations are
fed to the device pre-transposed ([E, L]) and pre-tiled so every DMA
moves >=4KB contiguous per partition.
"""

import sys

sys.path.insert(0, "/opt/trn_rl_repo")

from contextlib import ExitStack

import numpy as np
import ml_dtypes

import concourse.bass as bass
import concourse.mybir as mybir
import concourse.tile as tile
from concourse import bacc
from concourse.bass_utils import run_bass_kernel_spmd

BF16 = mybir.dt.bfloat16
F32 = mybir.dt.float32

B, L, E, H, D = 4, 2048, 1024, 16, 64
NCORES = 8
HPC = H // 2          # heads per core (8)
DH = HPC * D          # per-core projected dim (512)
LB = 512              # q-block width
NLB = L // LB         # 4
ET = E // 128         # 8 contraction tiles for projections
EG = 2                # e-tile groups per DMA (ET // 4)
MT = DH // 128        # 4 dout tiles
KT = L // 128         # 16 key tiles
EXP_SCALE = 1.0 / np.sqrt(D)


def _build():
    nc = bacc.Bacc("TRN2", target_bir_lowering=False, debug=False,
                   num_devices=NCORES)

    qT_in = nc.dram_tensor("qT", [EG, NLB, 128, ET // EG, LB], BF16, kind="ExternalInput").ap()
    kT_in = nc.dram_tensor("kT", [EG, NLB, 128, ET // EG, LB], BF16, kind="ExternalInput").ap()
    vT_in = nc.dram_tensor("vT", [EG, NLB, 128, ET // EG, LB], BF16, kind="ExternalInput").ap()
    wq_in = nc.dram_tensor("wq", [128, ET, DH], BF16, kind="ExternalInput").ap()
    wk_in = nc.dram_tensor("wk", [128, ET, DH], BF16, kind="ExternalInput").ap()
    wv_in = nc.dram_tensor("wv", [128, ET, DH], BF16, kind="ExternalInput").ap()
    wo_in = nc.dram_tensor("wo", [128, MT, E], BF16, kind="ExternalInput").ap()
    bias_in = nc.dram_tensor("bias", [128, KT], F32, kind="ExternalInput").ap()
    out_ext = nc.dram_tensor("out", [KT, 128, 2, LB], BF16, kind="ExternalOutput").ap()

    with tile.TileContext(nc) as tc, ExitStack() as ctx:
        wpool = ctx.enter_context(tc.tile_pool(name="weights", bufs=1))
        ppool = ctx.enter_context(tc.tile_pool(name="persist", bufs=1))
        xpool = ctx.enter_context(tc.tile_pool(name="xT", bufs=3))
        pTpool = ctx.enter_context(tc.tile_pool(name="pT", bufs=3))
        opool = ctx.enter_context(tc.tile_pool(name="outsb", bufs=3))
        rpool = ctx.enter_context(tc.tile_pool(name="rnorm", bufs=4))
        ps_proj = ctx.enter_context(tc.tile_pool(name="ps_proj", bufs=2, space="PSUM"))
        ps_sp = ctx.enter_context(tc.tile_pool(name="ps_sp", bufs=2, space="PSUM"))
        ps_oacc = ctx.enter_context(tc.tile_pool(name="ps_oacc", bufs=2, space="PSUM"))

        # ---- resident weights (one DMA each, >=4KB/partition) ------------
        wq_sb = wpool.tile([128, ET, DH], BF16, tag="wq")
        wk_sb = wpool.tile([128, ET, DH], BF16, tag="wk")
        wv_sb = wpool.tile([128, ET, DH], BF16, tag="wv")
        wo_sb = wpool.tile([128, MT, E], BF16, tag="wo")
        nc.sync.dma_start(wq_sb[:], wq_in[:])
        nc.sync.dma_start(wk_sb[:], wk_in[:])
        nc.sync.dma_start(wv_sb[:], wv_in[:])
        nc.sync.dma_start(wo_sb[:], wo_in[:])
        bias_sb = wpool.tile([128, KT], F32, tag="bias")
        nc.sync.dma_start(bias_sb[:], bias_in[:])

        # ---- resident activations ---------------------------------------
        qT_sb = ppool.tile([128, MT, L], BF16, tag="qT")
        kT_sb = ppool.tile([128, MT, L], BF16, tag="kT")
        v_sb = ppool.tile([128, KT, HPC, D + 1], BF16, tag="v")
        oT_sb = ppool.tile([128, MT, L], BF16, tag="oT")
        nc.gpsimd.memset(v_sb[:, :, :, D:D + 1], 1.0)

        # ---- pipeline units ---------------------------------------------
        def proj_block(lb):
            xq, xk, xv = [], [], []
            for eg in range(EG):
                tq = xpool.tile([128, ET // EG, LB], BF16, tag="xq")
                nc.sync.dma_start(tq[:], qT_in[eg, lb])
                xq.append(tq)
                tk = xpool.tile([128, ET // EG, LB], BF16, tag="xk")
                nc.sync.dma_start(tk[:], kT_in[eg, lb])
                xk.append(tk)
                tv = xpool.tile([128, ET // EG, LB], BF16, tag="xv")
                nc.sync.dma_start(tv[:], vT_in[eg, lb])
                xv.append(tv)
            for dst, xs, w_sb in ((qT_sb, xq, wq_sb), (kT_sb, xk, wk_sb)):
                for m in range(MT):
                    ps = ps_proj.tile([128, LB], F32, tag="ps_proj")
                    for t in range(ET):
                        nc.tensor.matmul(
                            ps[:],
                            lhsT=w_sb[:, t, m * 128:(m + 1) * 128],
                            rhs=xs[t // 4][:, t % 4, :],
                            start=(t == 0), stop=(t == ET - 1))
                    nc.vector.tensor_copy(dst[:, m, lb * LB:(lb + 1) * LB], ps[:])
            for lt in range(LB // 128):
                ps = ps_proj.tile([128, HPC, D], F32, tag="ps_proj")
                for t in range(ET):
                    nc.tensor.matmul(
                        ps[:],
                        lhsT=xv[t // 4][:, t % 4, lt * 128:(lt + 1) * 128],
                        rhs=wv_sb[:, t, :],
                        start=(t == 0), stop=(t == ET - 1))
                nc.vector.tensor_copy(v_sb[:, lb * 4 + lt, :, 0:D], ps[:])

        def attn_pair(hp, j):
            nki = 4 * j + 4
            oaccs = [ps_oacc.tile([D + 1, LB], F32, tag="oacc", name="oacc")
                     for _ in range(2)]
            for ki in range(nki):
                s = ki - 4 * j
                x0 = 128 * s if s >= 0 else 0   # first causal-valid q column
                sp = ps_sp.tile([128, 2, LB], F32, tag="sp")
                for hi in range(2):
                    p0 = hi * 64
                    nc.tensor.matmul(
                        sp[:, hi, x0:LB],
                        lhsT=kT_sb[p0:p0 + 64, hp, ki * 128:(ki + 1) * 128],
                        rhs=qT_sb[p0:p0 + 64, hp, j * LB + x0:(j + 1) * LB],
                        start=True, stop=True, tile_position=(p0, 0))
                pT = pTpool.tile([128, 2, LB], BF16, tag="pT")
                nc.scalar.activation(pT[:, :, x0:LB], sp[:, :, x0:LB],
                                     mybir.ActivationFunctionType.Exp,
                                     bias=bias_sb[:, ki:ki + 1],
                                     scale=float(EXP_SCALE))
                if s >= 0:
                    # zero q < k inside the first 128 valid columns
                    nc.gpsimd.affine_select(
                        out=pT[:, :, x0:x0 + 128], in_=pT[:, :, x0:x0 + 128],
                        compare_op=mybir.AluOpType.is_ge,
                        fill=0.0,
                        base=0,
                        pattern=[[0, 2], [1, 128]],
                        channel_multiplier=-1)
                for hi in range(2):
                    nc.tensor.matmul(
                        oaccs[hi][:, x0:LB],
                        lhsT=v_sb[:, ki, 2 * hp + hi, :],
                        rhs=pT[:, hi, x0:LB],
                        start=(ki == 0), stop=(ki == nki - 1))
            osbs = []
            for hi in range(2):
                osb = rpool.tile([D + 1, LB], F32, tag="osb", name="osb")
                nc.vector.tensor_copy(osb[:], oaccs[hi][:])
                osbs.append(osb)

            def norm_fn():
                for hi in range(2):
                    osb = osbs[hi]
                    rsum = rpool.tile([1, LB], F32, tag="rsum")
                    nc.vector.tensor_copy(rsum[:], osb[D:D + 1, :])
                    rinv1 = rpool.tile([1, LB], F32, tag="rinv1")
                    nc.vector.reciprocal_approx_fast(rinv1[:], rsum[:])
                    rinv = rpool.tile([64, LB], F32, tag="rinv")
                    nc.gpsimd.partition_broadcast(rinv[:], rinv1[:])
                    p0 = hi * 64
                    nc.vector.tensor_mul(
                        oT_sb[p0:p0 + 64, hp, j * LB:(j + 1) * LB],
                        osb[0:D, :], rinv[:])
            return norm_fn

        def oproj_block(lb):
            for lt in range(LB // 128):
                l_tile = lb * 4 + lt
                ob = opool.tile([128, 2, LB], BF16, tag="ob")
                for e in range(2):
                    ps = ps_proj.tile([128, LB], F32, tag="ps_proj")
                    for r in range(MT):
                        nc.tensor.matmul(
                            ps[:],
                            lhsT=oT_sb[:, r, l_tile * 128:(l_tile + 1) * 128],
                            rhs=wo_sb[:, r, e * LB:(e + 1) * LB],
                            start=(r == 0), stop=(r == MT - 1))
                    nc.vector.tensor_copy(ob[:, e, :], ps[:])
                nc.sync.dma_start(out_ext[l_tile], ob[:])

        proj_block(0)
        pending_norm = None
        for j in range(NLB):
            for hp in range(HPC // 2):
                nf = attn_pair(hp, j)
                if pending_norm is not None:
                    pending_norm()
                pending_norm = nf
            # flush before oproj needs oT of this j-block
            pending_norm()
            pending_norm = None
            if j + 1 < NLB:
                proj_block(j + 1)
            oproj_block(j)

    nc.compile()
    return nc


_CACHE = {}


def _get_nc():
    if "nc" not in _CACHE:
        _CACHE["nc"] = _build()
    return _CACHE["nc"]


def _prepare_in_maps(query, key, value, pad_mask, Wq, Wk, Wv, Wo):
    bf = ml_dtypes.bfloat16
    query = np.asarray(query, np.float32)
    key = np.asarray(key, np.float32)
    value = np.asarray(value, np.float32)
    pad_mask = np.asarray(pad_mask)
    Wq = np.asarray(Wq, np.float32)
    Wk = np.asarray(Wk, np.float32)
    Wv = np.asarray(Wv, np.float32)
    Wo = np.asarray(Wo, np.float32)

    def tile_act(x):
        # [L, E] -> [E, L] -> [EG, NLB, 128, ET//EG, LB], 4KB/partition chunks
        xt = x.T.reshape(EG, ET // EG, 128, NLB, LB).transpose(0, 3, 2, 1, 4)
        return np.ascontiguousarray(xt.astype(bf))

    per_batch = []
    for b in range(B):
        bias = np.where(pad_mask[b] != 0, 0.0, -30000.0).astype(np.float32)
        bias = np.ascontiguousarray(bias.reshape(KT, 128).T)
        per_batch.append({
            "qT": tile_act(query[b]),
            "kT": tile_act(key[b]),
            "vT": tile_act(value[b]),
            "bias": bias,
        })

    per_group = []
    for g in range(2):
        sl = slice(g * DH, (g + 1) * DH)
        per_group.append({
            "wq": np.ascontiguousarray(
                Wq[:, sl].astype(bf).reshape(ET, 128, DH).transpose(1, 0, 2)),
            "wk": np.ascontiguousarray(
                Wk[:, sl].astype(bf).reshape(ET, 128, DH).transpose(1, 0, 2)),
            "wv": np.ascontiguousarray(
                Wv[:, sl].astype(bf).reshape(ET, 128, DH).transpose(1, 0, 2)),
            "wo": np.ascontiguousarray(
                Wo[sl, :].astype(bf).reshape(MT, 128, E).transpose(1, 0, 2)),
        })

    in_maps = []
    for b in range(B):
        for g in range(2):
            m = dict(per_batch[b])
            m.update(per_group[g])
            in_maps.append(m)
    return in_maps


def _combine(results):
    out = np.empty((B, L, E), np.float32)
    for b in range(B):
        acc = (results[2 * b]["out"].astype(np.float32)
               + results[2 * b + 1]["out"].astype(np.float32))
        out[b] = acc.reshape(L, E)
    return out


def kernel(query, key, value, pad_mask, Wq, Wk, Wv, Wo):
    nc = _get_nc()
    in_maps = _prepare_in_maps(query, key, value, pad_mask, Wq, Wk, Wv, Wo)
    res = run_bass_kernel_spmd(nc, in_maps, core_ids=list(range(NCORES)))
    return _combine(res.results)


# revision 13
# speedup vs baseline: 1.4743x; 1.0461x over previous
"""Causal multi-head attention on 8 TRN2 NeuronCores.

Sharding: 8 cores = 4 batches x 2 head-groups (8 heads each).
Each core computes q/k/v projections for its head group, flash-style
causal attention in S^T layout ([k, q], softmax across partitions via a
ones-column in the PV matmul), and a partial output projection
(row-split Wo).  Host sums the two partial outputs per batch.

All matmuls run in bf16 with fp32 PSUM accumulation.  Activations are
fed to the device pre-transposed ([E, L]) and pre-tiled so every DMA
moves >=4KB contiguous per partition.
"""

import sys

sys.path.insert(0, "/opt/trn_rl_repo")

from contextlib import ExitStack

import numpy as np
import ml_dtypes

import concourse.bass as bass
import concourse.mybir as mybir
import concourse.tile as tile
from concourse import bacc
from concourse.bass_utils import run_bass_kernel_spmd

BF16 = mybir.dt.bfloat16
F32 = mybir.dt.float32

B, L, E, H, D = 4, 2048, 1024, 16, 64
NCORES = 8
HPC = H // 2          # heads per core (8)
DH = HPC * D          # per-core projected dim (512)
LB = 512              # q-block width
NLB = L // LB         # 4
ET = E // 128         # 8 contraction tiles for projections
EG = 2                # e-tile groups per DMA (ET // 4)
MT = DH // 128        # 4 dout tiles
KT = L // 128         # 16 key tiles
EXP_SCALE = 1.0 / np.sqrt(D)


def _build():
    nc = bacc.Bacc("TRN2", target_bir_lowering=False, debug=False,
                   num_devices=NCORES)

    qT_in = nc.dram_tensor("qT", [EG, NLB, 128, ET // EG, LB], BF16, kind="ExternalInput").ap()
    kT_in = nc.dram_tensor("kT", [EG, NLB, 128, ET // EG, LB], BF16, kind="ExternalInput").ap()
    vT_in = nc.dram_tensor("vT", [EG, NLB, 128, ET // EG, LB], BF16, kind="ExternalInput").ap()
    wq_in = nc.dram_tensor("wq", [128, ET, DH], BF16, kind="ExternalInput").ap()
    wk_in = nc.dram_tensor("wk", [128, ET, DH], BF16, kind="ExternalInput").ap()
    wv_in = nc.dram_tensor("wv", [128, ET, DH], BF16, kind="ExternalInput").ap()
    wo_in = nc.dram_tensor("wo", [128, MT, E], BF16, kind="ExternalInput").ap()
    bias_in = nc.dram_tensor("bias", [128, KT], F32, kind="ExternalInput").ap()
    out_ext = nc.dram_tensor("out", [KT, 128, 2, LB], BF16, kind="ExternalOutput").ap()

    with tile.TileContext(nc) as tc, ExitStack() as ctx:
        wpool = ctx.enter_context(tc.tile_pool(name="weights", bufs=1))
        ppool = ctx.enter_context(tc.tile_pool(name="persist", bufs=1))
        xpool = ctx.enter_context(tc.tile_pool(name="xT", bufs=3))
        pTpool = ctx.enter_context(tc.tile_pool(name="pT", bufs=3))
        opool = ctx.enter_context(tc.tile_pool(name="outsb", bufs=3))
        rpool = ctx.enter_context(tc.tile_pool(name="rnorm", bufs=4))
        ps_proj = ctx.enter_context(tc.tile_pool(name="ps_proj", bufs=2, space="PSUM"))
        ps_sp = ctx.enter_context(tc.tile_pool(name="ps_sp", bufs=2, space="PSUM"))
        ps_oacc = ctx.enter_context(tc.tile_pool(name="ps_oacc", bufs=2, space="PSUM"))

        # ---- resident weights (one DMA each, >=4KB/partition) ------------
        wq_sb = wpool.tile([128, ET, DH], BF16, tag="wq")
        wk_sb = wpool.tile([128, ET, DH], BF16, tag="wk")
        wv_sb = wpool.tile([128, ET, DH], BF16, tag="wv")
        wo_sb = wpool.tile([128, MT, E], BF16, tag="wo")
        nc.sync.dma_start(wq_sb[:], wq_in[:])
        nc.sync.dma_start(wk_sb[:], wk_in[:])
        nc.sync.dma_start(wv_sb[:], wv_in[:])
        nc.sync.dma_start(wo_sb[:], wo_in[:])
        bias_sb = wpool.tile([128, KT], F32, tag="bias")
        nc.sync.dma_start(bias_sb[:], bias_in[:])

        # ---- resident activations ---------------------------------------
        qT_sb = ppool.tile([128, MT, L], BF16, tag="qT")
        kT_sb = ppool.tile([128, MT, L], BF16, tag="kT")
        v_sb = ppool.tile([128, KT, HPC, D + 1], BF16, tag="v")
        oT_sb = ppool.tile([128, MT, L], BF16, tag="oT")
        nc.gpsimd.memset(v_sb[:, :, :, D:D + 1], 1.0)

        # ---- pipeline units ---------------------------------------------
        xcache = {}

        def get_xtiles(lb):
            if lb not in xcache:
                xq, xk, xv = [], [], []
                for eg in range(EG):
                    tq = xpool.tile([128, ET // EG, LB], BF16, tag="xq", name="xq")
                    nc.sync.dma_start(tq[:], qT_in[eg, lb])
                    xq.append(tq)
                    tk = xpool.tile([128, ET // EG, LB], BF16, tag="xk", name="xk")
                    nc.sync.dma_start(tk[:], kT_in[eg, lb])
                    xk.append(tk)
                    tv = xpool.tile([128, ET // EG, LB], BF16, tag="xv", name="xv")
                    nc.sync.dma_start(tv[:], vT_in[eg, lb])
                    xv.append(tv)
                xcache[lb] = (xq, xk, xv)
            return xcache[lb]

        def proj_qk_unit(lb, which, m):
            def fn():
                xq, xk, xv = get_xtiles(lb)
                dst = (qT_sb, kT_sb)[which]
                xs = (xq, xk)[which]
                w_sb = (wq_sb, wk_sb)[which]
                ps = ps_proj.tile([128, LB], F32, tag="ps_proj", name="psp")
                for t in range(ET):
                    nc.tensor.matmul(
                        ps[:],
                        lhsT=w_sb[:, t, m * 128:(m + 1) * 128],
                        rhs=xs[t // 4][:, t % 4, :],
                        start=(t == 0), stop=(t == ET - 1))
                nc.vector.tensor_copy(dst[:, m, lb * LB:(lb + 1) * LB], ps[:])
            return fn

        def proj_v_unit(lb, lt):
            def fn():
                xq, xk, xv = get_xtiles(lb)
                ps = ps_proj.tile([128, HPC, D], F32, tag="ps_proj", name="psv")
                for t in range(ET):
                    nc.tensor.matmul(
                        ps[:],
                        lhsT=xv[t // 4][:, t % 4, lt * 128:(lt + 1) * 128],
                        rhs=wv_sb[:, t, :],
                        start=(t == 0), stop=(t == ET - 1))
                nc.vector.tensor_copy(v_sb[:, lb * 4 + lt, :, 0:D], ps[:])
            return fn

        def oproj_unit(lb, lt):
            def fn():
                l_tile = lb * 4 + lt
                ob = opool.tile([128, 2, LB], BF16, tag="ob", name="ob")
                for e in range(2):
                    ps = ps_proj.tile([128, LB], F32, tag="ps_proj", name="pso")
                    for r in range(MT):
                        nc.tensor.matmul(
                            ps[:],
                            lhsT=oT_sb[:, r, l_tile * 128:(l_tile + 1) * 128],
                            rhs=wo_sb[:, r, e * LB:(e + 1) * LB],
                            start=(r == 0), stop=(r == MT - 1))
                    nc.vector.tensor_copy(ob[:, e, :], ps[:])
                nc.sync.dma_start(out_ext[l_tile], ob[:])
            return fn

        def proj_block(lb):
            for which in range(2):
                for m in range(MT):
                    proj_qk_unit(lb, which, m)()
            for lt in range(LB // 128):
                proj_v_unit(lb, lt)()

        def attn_pair(hp, j, filler):
            nki = 4 * j + 4
            oaccs = [ps_oacc.tile([D + 1, LB], F32, tag="oacc", name="oacc")
                     for _ in range(2)]
            for ki in range(nki):
                s = ki - 4 * j
                x0 = 128 * s if s >= 0 else 0   # first causal-valid q column
                sp = ps_sp.tile([128, 2, LB], F32, tag="sp", name="sp")
                for hi in range(2):
                    p0 = hi * 64
                    nc.tensor.matmul(
                        sp[:, hi, x0:LB],
                        lhsT=kT_sb[p0:p0 + 64, hp, ki * 128:(ki + 1) * 128],
                        rhs=qT_sb[p0:p0 + 64, hp, j * LB + x0:(j + 1) * LB],
                        start=True, stop=True, tile_position=(p0, 0))
                pT = pTpool.tile([128, 2, LB], BF16, tag="pT", name="pT")
                nc.scalar.activation(pT[:, :, x0:LB], sp[:, :, x0:LB],
                                     mybir.ActivationFunctionType.Exp,
                                     bias=bias_sb[:, ki:ki + 1],
                                     scale=float(EXP_SCALE))
                if s >= 0:
                    # zero q < k inside the first 128 valid columns
                    nc.gpsimd.affine_select(
                        out=pT[:, :, x0:x0 + 128], in_=pT[:, :, x0:x0 + 128],
                        compare_op=mybir.AluOpType.is_ge,
                        fill=0.0,
                        base=0,
                        pattern=[[0, 2], [1, 128]],
                        channel_multiplier=-1)
                for hi in range(2):
                    nc.tensor.matmul(
                        oaccs[hi][:, x0:LB],
                        lhsT=v_sb[:, ki, 2 * hp + hi, :],
                        rhs=pT[:, hi, x0:LB],
                        start=(ki == 0), stop=(ki == nki - 1))
                filler()
            osbs = []
            for hi in range(2):
                osb = rpool.tile([D + 1, LB], F32, tag="osb", name="osb")
                nc.vector.tensor_copy(osb[:], oaccs[hi][:])
                osbs.append(osb)

            def norm_fn():
                for hi in range(2):
                    osb = osbs[hi]
                    rsum = rpool.tile([1, LB], F32, tag="rsum", name="rsum")
                    nc.vector.tensor_copy(rsum[:], osb[D:D + 1, :])
                    rinv1 = rpool.tile([1, LB], F32, tag="rinv1", name="rinv1")
                    nc.vector.reciprocal_approx_fast(rinv1[:], rsum[:])
                    rinv = rpool.tile([64, LB], F32, tag="rinv", name="rinv")
                    nc.gpsimd.partition_broadcast(rinv[:], rinv1[:])
                    p0 = hi * 64
                    nc.vector.tensor_mul(
                        oT_sb[p0:p0 + 64, hp, j * LB:(j + 1) * LB],
                        osb[0:D, :], rinv[:])
            return norm_fn

        proj_block(0)
        pending_norm = None
        for j in range(NLB):
            units = []
            if j + 1 < NLB:
                for m in range(MT):
                    units.append(proj_qk_unit(j + 1, 0, m))
                for m in range(MT):
                    units.append(proj_qk_unit(j + 1, 1, m))
                for lt in range(LB // 128):
                    units.append(proj_v_unit(j + 1, lt))
            if j >= 1:
                for lt in range(LB // 128):
                    units.append(oproj_unit(j - 1, lt))

            state = {"slot": 0, "done": 0}
            total_slots = (HPC // 2) * (4 * j + 4)

            def filler():
                state["slot"] += 1
                want = len(units) * state["slot"] // total_slots
                while state["done"] < want:
                    units[state["done"]]()
                    state["done"] += 1

            for hp in range(HPC // 2):
                nf = attn_pair(hp, j, filler)
                if pending_norm is not None:
                    pending_norm()
                pending_norm = nf
            while state["done"] < len(units):
                units[state["done"]]()
                state["done"] += 1
            pending_norm()
            pending_norm = None
        for lt in range(LB // 128):
            oproj_unit(NLB - 1, lt)()

    nc.compile()
    return nc


_CACHE = {}


def _get_nc():
    if "nc" not in _CACHE:
        _CACHE["nc"] = _build()
    return _CACHE["nc"]


def _prepare_in_maps(query, key, value, pad_mask, Wq, Wk, Wv, Wo):
    bf = ml_dtypes.bfloat16
    query = np.asarray(query, np.float32)
    key = np.asarray(key, np.float32)
    value = np.asarray(value, np.float32)
    pad_mask = np.asarray(pad_mask)
    Wq = np.asarray(Wq, np.float32)
    Wk = np.asarray(Wk, np.float32)
    Wv = np.asarray(Wv, np.float32)
    Wo = np.asarray(Wo, np.float32)

    def tile_act(x):
        # [L, E] -> [E, L] -> [EG, NLB, 128, ET//EG, LB], 4KB/partition chunks
        xt = x.T.reshape(EG, ET // EG, 128, NLB, LB).transpose(0, 3, 2, 1, 4)
        return np.ascontiguousarray(xt.astype(bf))

    per_batch = []
    for b in range(B):
        bias = np.where(pad_mask[b] != 0, 0.0, -30000.0).astype(np.float32)
        bias = np.ascontiguousarray(bias.reshape(KT, 128).T)
        per_batch.append({
            "qT": tile_act(query[b]),
            "kT": tile_act(key[b]),
            "vT": tile_act(value[b]),
            "bias": bias,
        })

    per_group = []
    for g in range(2):
        sl = slice(g * DH, (g + 1) * DH)
        per_group.append({
            "wq": np.ascontiguousarray(
                Wq[:, sl].astype(bf).reshape(ET, 128, DH).transpose(1, 0, 2)),
            "wk": np.ascontiguousarray(
                Wk[:, sl].astype(bf).reshape(ET, 128, DH).transpose(1, 0, 2)),
            "wv": np.ascontiguousarray(
                Wv[:, sl].astype(bf).reshape(ET, 128, DH).transpose(1, 0, 2)),
            "wo": np.ascontiguousarray(
                Wo[sl, :].astype(bf).reshape(MT, 128, E).transpose(1, 0, 2)),
        })

    in_maps = []
    for b in range(B):
        for g in range(2):
            m = dict(per_batch[b])
            m.update(per_group[g])
            in_maps.append(m)
    return in_maps


def _combine(results):
    out = np.empty((B, L, E), np.float32)
    for b in range(B):
        acc = (results[2 * b]["out"].astype(np.float32)
               + results[2 * b + 1]["out"].astype(np.float32))
        out[b] = acc.reshape(L, E)
    return out


def kernel(query, key, value, pad_mask, Wq, Wk, Wv, Wo):
    nc = _get_nc()
    in_maps = _prepare_in_maps(query, key, value, pad_mask, Wq, Wk, Wv, Wo)
    res = run_bass_kernel_spmd(nc, in_maps, core_ids=list(range(NCORES)))
    return _combine(res.results)


# revision 15
# speedup vs baseline: 1.5350x; 1.0412x over previous
"""Causal multi-head attention on 8 TRN2 NeuronCores.

Sharding: 8 cores = 4 batches x 2 head-groups (8 heads each).
Each core computes q/k/v projections for its head group, flash-style
causal attention in S^T layout ([k, q], softmax across partitions via a
ones-column in the PV matmul), and a partial output projection
(row-split Wo).  Host sums the two partial outputs per batch.

All matmuls run in bf16 with fp32 PSUM accumulation.  Activations are
fed to the device pre-transposed ([E, L]) and pre-tiled so every DMA
moves >=4KB contiguous per partition.
"""

import sys

sys.path.insert(0, "/opt/trn_rl_repo")

from contextlib import ExitStack

import numpy as np
import ml_dtypes

import concourse.bass as bass
import concourse.mybir as mybir
import concourse.tile as tile
from concourse import bacc
from concourse.bass_utils import run_bass_kernel_spmd

BF16 = mybir.dt.bfloat16
F32 = mybir.dt.float32
F8 = mybir.dt.float8e4

B, L, E, H, D = 4, 2048, 1024, 16, 64
NCORES = 8
HPC = H // 2          # heads per core (8)
DH = HPC * D          # per-core projected dim (512)
LB = 512              # q-block width
NLB = L // LB         # 4
ET = E // 128         # 8 contraction tiles for projections
EG = 2                # e-tile groups per DMA (ET // 4)
MT = DH // 128        # 4 dout tiles
KT = L // 128         # 16 key tiles
EXP_SCALE = 1.0 / np.sqrt(D)


def _build():
    nc = bacc.Bacc("TRN2", target_bir_lowering=False, debug=False,
                   num_devices=NCORES)

    qT_in = nc.dram_tensor("qT", [EG, NLB, 128, ET // EG, LB], BF16, kind="ExternalInput").ap()
    kT_in = nc.dram_tensor("kT", [EG, NLB, 128, ET // EG, LB], BF16, kind="ExternalInput").ap()
    vT_in = nc.dram_tensor("vT", [EG, NLB, 128, ET // EG, LB], BF16, kind="ExternalInput").ap()
    wq_in = nc.dram_tensor("wq", [128, ET, DH], BF16, kind="ExternalInput").ap()
    wk_in = nc.dram_tensor("wk", [128, ET, DH], BF16, kind="ExternalInput").ap()
    wv_in = nc.dram_tensor("wv", [128, ET, DH], BF16, kind="ExternalInput").ap()
    wo_in = nc.dram_tensor("wo", [128, MT, E], BF16, kind="ExternalInput").ap()
    bias_in = nc.dram_tensor("bias", [128, KT], F32, kind="ExternalInput").ap()
    out_ext = nc.dram_tensor("out", [KT, 128, 2, LB], BF16, kind="ExternalOutput").ap()

    with tile.TileContext(nc) as tc, ExitStack() as ctx:
        wpool = ctx.enter_context(tc.tile_pool(name="weights", bufs=1))
        ppool = ctx.enter_context(tc.tile_pool(name="persist", bufs=1))
        xpool = ctx.enter_context(tc.tile_pool(name="xT", bufs=3))
        pTpool = ctx.enter_context(tc.tile_pool(name="pT", bufs=3))
        opool = ctx.enter_context(tc.tile_pool(name="outsb", bufs=3))
        rpool = ctx.enter_context(tc.tile_pool(name="rnorm", bufs=4))
        ps_proj = ctx.enter_context(tc.tile_pool(name="ps_proj", bufs=2, space="PSUM"))
        ps_sp = ctx.enter_context(tc.tile_pool(name="ps_sp", bufs=2, space="PSUM"))
        ps_oacc = ctx.enter_context(tc.tile_pool(name="ps_oacc", bufs=2, space="PSUM"))

        # ---- resident weights (one DMA each, >=4KB/partition) ------------
        wq_sb = wpool.tile([128, ET, DH], BF16, tag="wq")
        wk_sb = wpool.tile([128, ET, DH], BF16, tag="wk")
        wv_sb = wpool.tile([128, ET, DH], BF16, tag="wv")
        wo_sb = wpool.tile([128, MT, E], BF16, tag="wo")
        nc.sync.dma_start(wq_sb[:], wq_in[:])
        nc.sync.dma_start(wk_sb[:], wk_in[:])
        nc.sync.dma_start(wv_sb[:], wv_in[:])
        nc.sync.dma_start(wo_sb[:], wo_in[:])
        bias_sb = wpool.tile([128, KT], F32, tag="bias")
        nc.sync.dma_start(bias_sb[:], bias_in[:])

        # ---- resident activations ---------------------------------------
        qT_sb = ppool.tile([128, MT, L], BF16, tag="qT")
        kT_sb = ppool.tile([128, MT, L], BF16, tag="kT")
        v_sb = ppool.tile([128, KT, HPC, D + 1], BF16, tag="v")
        oT_sb = ppool.tile([128, MT, L], BF16, tag="oT")
        nc.gpsimd.memset(v_sb[:, :, :, D:D + 1], 1.0)

        # ---- pipeline units ---------------------------------------------
        xcache = {}

        def get_xtiles(lb):
            if lb not in xcache:
                xq, xk, xv = [], [], []
                for eg in range(EG):
                    tq = xpool.tile([128, ET // EG, LB], BF16, tag="xq", name="xq")
                    nc.sync.dma_start(tq[:], qT_in[eg, lb])
                    xq.append(tq)
                    tk = xpool.tile([128, ET // EG, LB], BF16, tag="xk", name="xk")
                    nc.sync.dma_start(tk[:], kT_in[eg, lb])
                    xk.append(tk)
                    tv = xpool.tile([128, ET // EG, LB], BF16, tag="xv", name="xv")
                    nc.sync.dma_start(tv[:], vT_in[eg, lb])
                    xv.append(tv)
                xcache[lb] = (xq, xk, xv)
            return xcache[lb]

        def proj_qk_unit(lb, which, m):
            def fn():
                xq, xk, xv = get_xtiles(lb)
                dst = (qT_sb, kT_sb)[which]
                xs = (xq, xk)[which]
                w_sb = (wq_sb, wk_sb)[which]
                ps = ps_proj.tile([128, LB], F32, tag="ps_proj", name="psp")
                for t in range(ET):
                    nc.tensor.matmul(
                        ps[:],
                        lhsT=w_sb[:, t, m * 128:(m + 1) * 128],
                        rhs=xs[t // 4][:, t % 4, :],
                        start=(t == 0), stop=(t == ET - 1))
                nc.vector.tensor_copy(dst[:, m, lb * LB:(lb + 1) * LB], ps[:])
            return fn

        def proj_v_unit(lb, lt):
            def fn():
                xq, xk, xv = get_xtiles(lb)
                ps = ps_proj.tile([128, HPC, D], F32, tag="ps_proj", name="psv")
                for t in range(ET):
                    nc.tensor.matmul(
                        ps[:],
                        lhsT=xv[t // 4][:, t % 4, lt * 128:(lt + 1) * 128],
                        rhs=wv_sb[:, t, :],
                        start=(t == 0), stop=(t == ET - 1))
                nc.vector.tensor_copy(v_sb[:, lb * 4 + lt, :, 0:D], ps[:])
            return fn

        def oproj_unit(lb, lt):
            def fn():
                l_tile = lb * 4 + lt
                ob = opool.tile([128, 2, LB], BF16, tag="ob", name="ob")
                for e in range(2):
                    ps = ps_proj.tile([128, LB], F32, tag="ps_proj", name="pso")
                    for r in range(MT):
                        nc.tensor.matmul(
                            ps[:],
                            lhsT=oT_sb[:, r, l_tile * 128:(l_tile + 1) * 128],
                            rhs=wo_sb[:, r, e * LB:(e + 1) * LB],
                            start=(r == 0), stop=(r == MT - 1))
                    nc.vector.tensor_copy(ob[:, e, :], ps[:])
                nc.sync.dma_start(out_ext[l_tile], ob[:])
            return fn

        def proj_block(lb):
            for which in range(2):
                for m in range(MT):
                    proj_qk_unit(lb, which, m)()
            for lt in range(LB // 128):
                proj_v_unit(lb, lt)()

        def attn_pair(hp, j, filler):
            nki = 4 * j + 4
            oaccs = [ps_oacc.tile([D + 1, LB], F32, tag="oacc", name="oacc")
                     for _ in range(2)]
            for ki in range(nki):
                s = ki - 4 * j
                x0 = 128 * s if s >= 0 else 0   # first causal-valid q column
                sp = ps_sp.tile([128, 2, LB], F32, tag="sp", name="sp")
                for hi in range(2):
                    p0 = hi * 64
                    nc.tensor.matmul(
                        sp[:, hi, x0:LB],
                        lhsT=kT_sb[p0:p0 + 64, hp, ki * 128:(ki + 1) * 128],
                        rhs=qT_sb[p0:p0 + 64, hp, j * LB + x0:(j + 1) * LB],
                        start=True, stop=True, tile_position=(p0, 0))
                pT = pTpool.tile([128, 2, LB], BF16, tag="pT", name="pT")
                nc.scalar.activation(pT[:, :, x0:LB], sp[:, :, x0:LB],
                                     mybir.ActivationFunctionType.Exp,
                                     bias=bias_sb[:, ki:ki + 1],
                                     scale=float(EXP_SCALE))
                if s >= 0:
                    # zero q < k inside the first 128 valid columns
                    nc.gpsimd.affine_select(
                        out=pT[:, :, x0:x0 + 128], in_=pT[:, :, x0:x0 + 128],
                        compare_op=mybir.AluOpType.is_ge,
                        fill=0.0,
                        base=0,
                        pattern=[[0, 2], [1, 128]],
                        channel_multiplier=-1)
                for hi in range(2):
                    nc.tensor.matmul(
                        oaccs[hi][:, x0:LB],
                        lhsT=v_sb[:, ki, 2 * hp + hi, :],
                        rhs=pT[:, hi, x0:LB],
                        start=(ki == 0), stop=(ki == nki - 1))
                filler()
            osbs = []
            for hi in range(2):
                osb = rpool.tile([D + 1, LB], F32, tag="osb", name="osb")
                nc.vector.tensor_copy(osb[:], oaccs[hi][:])
                osbs.append(osb)

            def norm_fn():
                for hi in range(2):
                    osb = osbs[hi]
                    rsum = rpool.tile([1, LB], F32, tag="rsum", name="rsum")
                    nc.vector.tensor_copy(rsum[:], osb[D:D + 1, :])
                    rinv1 = rpool.tile([1, LB], F32, tag="rinv1", name="rinv1")
                    nc.vector.reciprocal_approx_fast(rinv1[:], rsum[:])
                    rinv = rpool.tile([64, LB], F32, tag="rinv", name="rinv")
                    nc.gpsimd.partition_broadcast(rinv[:], rinv1[:])
                    p0 = hi * 64
                    nc.vector.tensor_mul(
                        oT_sb[p0:p0 + 64, hp, j * LB:(j + 1) * LB],
                        osb[0:D, :], rinv[:])
            return norm_fn

        proj_block(0)
        pending_norm = None
        for j in range(NLB):
            units = []
            if j + 1 < NLB:
                for m in range(MT):
                    units.append(proj_qk_unit(j + 1, 0, m))
                for m in range(MT):
                    units.append(proj_qk_unit(j + 1, 1, m))
                for lt in range(LB // 128):
                    units.append(proj_v_unit(j + 1, lt))
            if j == 2:
                for lt in range(LB // 128):
                    units.append(oproj_unit(0, lt))
            if j == 3:
                for lt in range(LB // 128):
                    units.append(oproj_unit(1, lt))
                for lt in range(LB // 128):
                    units.append(oproj_unit(2, lt))

            state = {"slot": 0, "done": 0}
            total_slots = (HPC // 2) * (4 * j + 4)

            def filler():
                state["slot"] += 1
                want = len(units) * state["slot"] // total_slots
                while state["done"] < want:
                    units[state["done"]]()
                    state["done"] += 1

            for hp in range(HPC // 2):
                nf = attn_pair(hp, j, filler)
                if pending_norm is not None:
                    pending_norm()
                pending_norm = nf
            while state["done"] < len(units):
                units[state["done"]]()
                state["done"] += 1
            pending_norm()
            pending_norm = None
        for lt in range(LB // 128):
            oproj_unit(NLB - 1, lt)()

    nc.compile()
    return nc


_CACHE = {}


def _get_nc():
    if "nc" not in _CACHE:
        _CACHE["nc"] = _build()
    return _CACHE["nc"]


def _prepare_in_maps(query, key, value, pad_mask, Wq, Wk, Wv, Wo):
    bf = ml_dtypes.bfloat16
    f8 = ml_dtypes.float8_e4m3fn
    query = np.asarray(query, np.float32)
    key = np.asarray(key, np.float32)
    value = np.asarray(value, np.float32)
    pad_mask = np.asarray(pad_mask)
    Wq = np.asarray(Wq, np.float32)
    Wk = np.asarray(Wk, np.float32)
    Wv = np.asarray(Wv, np.float32)
    Wo = np.asarray(Wo, np.float32)

    def tile_act(x):
        # [L, E] -> [E, L] -> [EG, NLB, 128, ET//EG, LB], 4KB/partition chunks
        xt = x.T.reshape(EG, ET // EG, 128, NLB, LB).transpose(0, 3, 2, 1, 4)
        return np.ascontiguousarray(xt.astype(bf))

    per_batch = []
    for b in range(B):
        bias = np.where(pad_mask[b] != 0, 0.0, -30000.0).astype(np.float32)
        bias = np.ascontiguousarray(bias.reshape(KT, 128).T)
        per_batch.append({
            "qT": tile_act(query[b]),
            "kT": tile_act(key[b]),
            "vT": tile_act(value[b]),
            "bias": bias,
        })

    per_group = []
    for g in range(2):
        sl = slice(g * DH, (g + 1) * DH)
        per_group.append({
            "wq": np.ascontiguousarray(
                Wq[:, sl].astype(bf).reshape(ET, 128, DH).transpose(1, 0, 2)),
            "wk": np.ascontiguousarray(
                Wk[:, sl].astype(bf).reshape(ET, 128, DH).transpose(1, 0, 2)),
            "wv": np.ascontiguousarray(
                Wv[:, sl].astype(bf).reshape(ET, 128, DH).transpose(1, 0, 2)),
            "wo": np.ascontiguousarray(
                Wo[sl, :].astype(bf).reshape(MT, 128, E).transpose(1, 0, 2)),
        })

    in_maps = []
    for b in range(B):
        for g in range(2):
            m = dict(per_batch[b])
            m.update(per_group[g])
            in_maps.append(m)
    return in_maps


def _combine(results):
    out = np.empty((B, L, E), np.float32)
    for b in range(B):
        acc = (results[2 * b]["out"].astype(np.float32)
               + results[2 * b + 1]["out"].astype(np.float32))
        out[b] = acc.reshape(L, E)
    return out


def kernel(query, key, value, pad_mask, Wq, Wk, Wv, Wo):
    nc = _get_nc()
    in_maps = _prepare_in_maps(query, key, value, pad_mask, Wq, Wk, Wv, Wo)
    res = run_bass_kernel_spmd(nc, in_maps, core_ids=list(range(NCORES)))
    return _combine(res.results)
